# revision 31
# baseline (speedup 1.0000x reference)
"""LocalSelfAttention (window=7) Trainium2 Bass kernel — pipelined v2.

Full inputs in, full output out. Sharding: 8 cores = batch(4) x seq-half(2),
each core handles 1024 tokens with a 3-token zero-padded halo on xs.

Math notes (exact rewrites of the reference):
- reference projects zero-PADDED xs patches, so out-of-range taps have
  k = b_ks, v = b_vs. Softmax over taps is invariant to the per-(t,h)
  constant q . b_ks, so the K bias drops entirely; softmax weights sum to 1,
  so the V bias contributes exactly b_vs to o. Both b_vs @ w_fc and b_fc are
  folded into the residual on the host: xq = x + b_vs @ w_fc + b_fc.

Structure (per core):
- Q projection (dc-outer, 8 psum banks) -> QT feature-major bf16.
- K projection over the 1056-wide halo in 3 stripes -> KT feature-major.
- Software-pipelined chunk loop over 11 chunks of 96 tokens:
  PE issue order interleaves score matmuls (chunk ci), V projection
  (chunk ci+1), prob-transposes + PV (chunk ci, delayed 3 pair-slots),
  and FC matmuls (128-token chunks, dependency-mapped) so the tensor
  engine never sits behind the softmax chain.  Elementwise work is spread
  over DVE (mask+scale stt, reduces, recip, pt-copy, FC residual-add),
  ACT (exp, V/OT evictions, LN squares + final scale), and Pool/GpSimd
  (prob normalization, LN scalar chain) - Pool cannot touch PSUM.
"""

import sys

for _p in ("/opt/trn_rl_repo",):
    if _p not in sys.path:
        sys.path.insert(0, _p)

import numpy as np
import ml_dtypes

BF16 = ml_dtypes.bfloat16

H, DK, DV, D = 16, 64, 64, 1024
NEI = 3
TEMP = 8.0
EPS = 1e-5
B, S = 4, 2048
NCORES = 8
T = (B * S) // NCORES          # 1024 tokens per core
TH = T + 2 * NEI               # 1030 halo tokens
P = 128
NT = T // P                    # 8 fc-phase token chunks (128 tokens)
ND = D // P                    # 8 feature chunks
CL = 96                        # attention chunk length
NCH = 11
CST = [96 * i for i in range(10)] + [928]          # attn chunk starts
TH2 = 1056                     # padded halo width
WB = 104                       # live score width (CL + 6 window, padded)
NEG = -30000.0
RSQRT_MAGIC = 0x5F3759DF       # Quake rsqrt seed; 2 Newton steps follow
KSTRIPES = [(0, 384), (384, 384), (768, 288)]      # K projection stripes
# FC chunk c (tokens 128c..128c+128) is emitted during attn chunk FCMAP^-1:
# dep(c) = first attn chunk index ci with 96ci+96 >= 128c+128; emit at dep+1.
FCMAP = {2: [0], 3: [1], 4: [2], 6: [3], 7: [4], 8: [5],
         10: [6, (896, 64)]}   # ci -> fc chunks; tuple = (t0, rows)
OFF = 5                        # pair-slots between scores and transpose/PV

_CACHE = {}


def _build_program(apply_affine: bool):
    import concourse.bacc as bacc
    import concourse.tile as tile
    from concourse import mybir
    from contextlib import ExitStack

    f32 = mybir.dt.float32
    bf16 = mybir.dt.bfloat16
    Alu = mybir.AluOpType
    Act = mybir.ActivationFunctionType

    nc = bacc.Bacc(
        "TRN2", target_bir_lowering=False, debug=False, enable_asserts=False
    )

    def din(name, shape, dt_):
        return nc.dram_tensor(name, shape, dt_, kind="ExternalInput").ap()

    xq = din("xq", (T, D), bf16)         # residual + folded fc/v bias
    xqT = din("xqT", (D, T), bf16)       # x^T (host-transposed)
    xsT = din("xsT", (D, TH), bf16)      # xs^T with halo (host-transposed)
    wq = din("wq", (D, D), bf16)
    wk = din("wk", (D, D), bf16)
    wv = din("wv", (D, D), bf16)
    wf = din("wf", (D, D), bf16)
    bq = din("bq", (P, ND), f32)         # b_qs laid out [p, ec]
    msk = din("msk", (CL, 2 * P), f32)   # band mask 0 / NEG, two head slots
    idn = din("idn", (CL, CL), bf16)     # identity for PE transpose
    if apply_affine:
        lng = din("lng", (1, D), f32)
        lnb = din("lnb", (1, D), f32)
    yo = nc.dram_tensor("yo", (T, D), bf16, kind="ExternalOutput").ap()

    with tile.TileContext(nc) as tc, ExitStack() as ctx:
        consts = ctx.enter_context(tc.tile_pool(name="consts", bufs=1))
        big = ctx.enter_context(tc.tile_pool(name="big", bufs=1))
        vpool = ctx.enter_context(tc.tile_pool(name="vpool", bufs=3))
        pmp = ctx.enter_context(tc.tile_pool(name="pmp", bufs=2))
        pep = ctx.enter_context(tc.tile_pool(name="pep", bufs=4))
        pnp = ctx.enter_context(tc.tile_pool(name="pnp", bufs=4))
        ptp = ctx.enter_context(tc.tile_pool(name="ptp", bufs=3))
        small = ctx.enter_context(tc.tile_pool(name="small", bufs=4))
        lnpool = ctx.enter_context(tc.tile_pool(name="lnpool", bufs=2))

        # ---- constants ----
        msk_sb = consts.tile([CL, 2 * P], f32, tag="msk")
        nc.sync.dma_start(out=msk_sb, in_=msk)
        idn_sb = consts.tile([CL, CL], bf16, tag="idn")
        nc.sync.dma_start(out=idn_sb, in_=idn)
        bq_sb = consts.tile([P, ND], f32, tag="bq")
        nc.sync.dma_start(out=bq_sb, in_=bq)
        magic_sb = consts.tile([P, 1], mybir.dt.uint32, tag="magic")
        nc.vector.memset(magic_sb, RSQRT_MAGIC)
        if apply_affine:
            import concourse.bass as bass

            g_bc = consts.tile([P, D], f32, tag="g_bc")
            b_bc = consts.tile([P, D], f32, tag="b_bc")
            nc.sync.dma_start(
                out=g_bc,
                in_=bass.AP(tensor=lng.tensor, offset=lng.offset,
                            ap=[[0, P]] + list(lng.ap[1:])),
            )
            nc.sync.dma_start(
                out=b_bc,
                in_=bass.AP(tensor=lnb.tensor, offset=lnb.offset,
                            ap=[[0, P]] + list(lnb.ap[1:])),
            )

        # ---- input loads: (wq,xT) pairs first so Q proj can start early ----
        wq_t, xT_t = [], []
        for dc in range(ND):
            w1 = big.tile([P, D], bf16, tag=f"wq{dc}", name=f"wq{dc}")
            nc.sync.dma_start(out=w1, in_=wq[dc * P:(dc + 1) * P, :])
            wq_t.append(w1)
            t1 = big.tile([P, T], bf16, tag=f"xT{dc}", name=f"xT{dc}")
            nc.sync.dma_start(out=t1, in_=xqT[dc * P:(dc + 1) * P, :])
            xT_t.append(t1)
        wk_t, xsT_t = [], []
        for dc in range(ND):
            w2 = big.tile([P, D], bf16, tag=f"wk{dc}", name=f"wk{dc}")
            nc.sync.dma_start(out=w2, in_=wk[dc * P:(dc + 1) * P, :])
            wk_t.append(w2)
            t2 = big.tile([P, TH2], bf16, tag=f"xsT{dc}", name=f"xsT{dc}")
            nc.sync.dma_start(out=t2[:, 0:TH], in_=xsT[dc * P:(dc + 1) * P, :])
            nc.vector.memset(t2[:, TH:TH2], 0.0)
            xsT_t.append(t2)
        wv_t, wf_t = [], []
        for dc in range(ND):
            w3 = big.tile([P, D], bf16, tag=f"wv{dc}", name=f"wv{dc}")
            nc.sync.dma_start(out=w3, in_=wv[dc * P:(dc + 1) * P, :])
            wv_t.append(w3)
        for dc in range(ND):
            w4 = big.tile([P, D], bf16, tag=f"wf{dc}", name=f"wf{dc}")
            nc.sync.dma_start(out=w4, in_=wf[dc * P:(dc + 1) * P, :])
            wf_t.append(w4)
        xr_t = []
        for c in range(NT):
            xr = big.tile([P, D], bf16, tag=f"xr{c}", name=f"xr{c}")
            nc.sync.dma_start(out=xr, in_=xq[c * P:(c + 1) * P, :])
            xr_t.append(xr)

        QT = [big.tile([P, T], bf16, tag=f"QT{ec}", name=f"QT{ec}")
              for ec in range(ND)]
        KT = [big.tile([P, TH2], bf16, tag=f"KT{ec}", name=f"KT{ec}")
              for ec in range(ND)]
        OT_all = big.tile([P, ND * T], bf16, tag="OT_all", name="OT_all")
        OTr = OT_all.rearrange("a (p w) -> a p w", p=ND)

        # ---- Q projection: dc-outer, 8 psum banks, ACT evict w/ bias ----
        with tc.tile_pool(name="psP", bufs=1, space="PSUM") as psP:
            for half in (0, 1):
                hs = slice(half * 512, (half + 1) * 512)
                ps = [psP.tile([P, 512], f32, tag=f"pp{ec}", name=f"pp{ec}")
                      for ec in range(ND)]
                for dc in range(ND):
                    for ec in range(ND):
                        nc.tensor.matmul(
                            ps[ec],
                            lhsT=wq_t[dc][:, ec * P:(ec + 1) * P],
                            rhs=xT_t[dc][:, hs],
                            start=(dc == 0), stop=(dc == ND - 1),
                        )
                for ec in range(ND):
                    nc.scalar.activation(out=QT[ec][:, hs], in_=ps[ec],
                                         func=Act.Identity,
                                         bias=bq_sb[:, ec:ec + 1], scale=1.0)
            # ---- K projection: 3 stripes over the 1056 halo, DVE evict ----
            for s0, w in KSTRIPES:
                ps = [psP.tile([P, 384], f32, tag=f"pp{ec}", name=f"pk{ec}")
                      for ec in range(ND)]
                for dc in range(ND):
                    for ec in range(ND):
                        nc.tensor.matmul(
                            ps[ec][:, 0:w],
                            lhsT=wk_t[dc][:, ec * P:(ec + 1) * P],
                            rhs=xsT_t[dc][:, s0:s0 + w],
                            start=(dc == 0), stop=(dc == ND - 1),
                        )
                for ec in range(ND):
                    nc.vector.tensor_copy(KT[ec][:, s0:s0 + w], ps[ec][:, 0:w])

        # ---- pipelined chunk loop ----
        with tc.tile_pool(name="psS", bufs=2, space="PSUM") as psS, \
             tc.tile_pool(name="psV", bufs=1, space="PSUM") as psV, \
             tc.tile_pool(name="psT", bufs=1, space="PSUM") as psT, \
             tc.tile_pool(name="psO", bufs=1, space="PSUM") as psO, \
             tc.tile_pool(name="psF", bufs=1, space="PSUM") as psF:

            V_tiles = {}

            def v_thunks(ci):
                """16 matmuls + 2 evictions projecting V for chunk ci."""
                s = CST[ci]
                vt = vpool.tile([P, D], bf16, tag="vt", name=f"V{ci}")
                V_tiles[ci] = vt
                out = []
                for half in (0, 1):
                    hs = slice(half * 512, (half + 1) * 512)
                    pv = [None]

                    def mk(dc, half=half, hs=hs, pv=pv, s=s, vt=vt):
                        def f():
                            if dc == 0:
                                pv[0] = psV.tile([P, 512], f32, tag="pv",
                                                 name="ps_v")
                            nc.tensor.matmul(
                                pv[0],
                                lhsT=xsT_t[dc][:, s:s + P],
                                rhs=wv_t[dc][:, hs],
                                start=(dc == 0), stop=(dc == ND - 1),
                            )
                            if dc == ND - 1:
                                nc.scalar.activation(out=vt[:, hs], in_=pv[0],
                                                     func=Act.Copy)
                        return f
                    out.extend(mk(dc) for dc in range(ND))
                return out

            def ln_block(cs, y_sb, ysums, rows=P):
                """LayerNorm for an FC token chunk from y_sb; + output DMA.

                rstd comes from a bit-trick seed + 2 Newton rsqrt steps on
                DVE/GpSimd - a Sqrt/Ln on ACT would leave the exp table and
                force a 1.3us ACT table reload.
                """
                def stile(tag, dt_=f32):
                    return small.tile([P, 1], dt_, tag=tag, name=tag)[0:rows]

                ysum = stile("ysum")
                nc.gpsimd.tensor_add(ysum, ysums[0], ysums[1])
                sq0 = stile("sq0")
                sq1 = stile("sq1")
                ysq = lnpool.tile([P, 512], f32, tag="ysq",
                                  name="ysq")[0:rows, :]
                nc.scalar.activation(out=ysq, in_=y_sb[:, 0:512],
                                     func=Act.Square, accum_out=sq0)
                nc.scalar.activation(out=ysq, in_=y_sb[:, 512:1024],
                                     func=Act.Square, accum_out=sq1)
                ssum = stile("ssum")
                nc.gpsimd.tensor_add(ssum, sq0, sq1)
                mean = stile("mean")
                nc.gpsimd.tensor_scalar_mul(mean, ysum, 1.0 / D)
                msq = stile("msq")
                nc.gpsimd.tensor_mul(msq, mean, mean)
                msqme = stile("msqme")
                nc.gpsimd.tensor_scalar_sub(msqme, msq, EPS)
                veps = stile("veps")
                nc.vector.scalar_tensor_tensor(
                    out=veps, in0=ssum, scalar=1.0 / D, in1=msqme,
                    op0=Alu.mult, op1=Alu.subtract,
                )
                u32 = mybir.dt.uint32
                sh_u = stile("sh_u", u32)
                nc.vector.tensor_scalar(
                    out=sh_u, in0=veps.bitcast(u32), scalar1=1, scalar2=None,
                    op0=Alu.logical_shift_right,
                )
                sb_u = stile("sb_u", u32)
                nc.vector.tensor_tensor(sb_u, magic_sb[0:rows], sh_u,
                                        Alu.subtract)
                r = sb_u.bitcast(f32)
                for it in range(2):
                    t1 = stile(f"nr{it}a")
                    nc.vector.tensor_mul(t1, r, r)
                    nc.vector.tensor_mul(t1, veps, t1)
                    nc.vector.tensor_scalar(
                        out=t1, in0=t1, scalar1=-0.5, scalar2=1.5,
                        op0=Alu.mult, op1=Alu.add,
                    )
                    rn = stile(f"nr{it}b")
                    nc.vector.tensor_mul(rn, r, t1)
                    r = rn
                rstd = r
                bact = stile("bact")
                nc.vector.scalar_tensor_tensor(
                    out=bact, in0=mean, scalar=-1.0, in1=rstd,
                    op0=Alu.mult, op1=Alu.mult,
                )
                out_sb = lnpool.tile([P, D], bf16, tag="osb",
                                     name="out_sb")[0:rows, :]
                nc.scalar.activation(out=out_sb, in_=y_sb, func=Act.Identity,
                                     bias=bact, scale=rstd)
                if apply_affine:
                    nc.vector.tensor_mul(out_sb, out_sb, g_bc[0:rows, :])
                    nc.vector.tensor_add(out_sb, out_sb, b_bc[0:rows, :])
                nc.sync.dma_start(out=yo[cs, :], in_=out_sb)

            def fc_thunks(c, t0=None, rows=P):
                """16 matmuls + 2 residual-stts + LN for an FC token chunk."""
                if t0 is None:
                    t0 = c * P
                cs = slice(t0, t0 + rows)
                y_sb = lnpool.tile([P, D], f32, tag="ysb", name="y_sb")[0:rows, :]
                ysums = [None, None]
                out = []
                for half in (0, 1):
                    hs = slice(half * 512, (half + 1) * 512)
                    pf = [None]

                    def mk(ec, half=half, hs=hs, pf=pf, cs=cs, y_sb=y_sb,
                           c=c):
                        def f():
                            if ec == 0:
                                pf[0] = psF.tile([P, 512], f32, tag="pf",
                                                 name="ps_f")[0:rows, :]
                            nc.tensor.matmul(
                                pf[0],
                                lhsT=OTr[:, ec, cs],
                                rhs=wf_t[ec][:, hs],
                                start=(ec == 0), stop=(ec == ND - 1),
                            )
                            if ec == ND - 1:
                                ysums[half] = small.tile(
                                    [P, 1], f32, tag=f"ysm{half}",
                                    name="ysm")[0:rows, :]
                                xrv = xr_t[t0 // P][t0 % P:t0 % P + rows, :]
                                nc.vector.scalar_tensor_tensor(
                                    out=y_sb[:, hs], in0=pf[0], scalar=1.0,
                                    in1=xrv[:, hs],
                                    op0=Alu.mult, op1=Alu.add,
                                    accum_out=ysums[half],
                                )
                        return f
                    out.extend(mk(ec) for ec in range(ND))
                out.append(lambda: ln_block(cs, y_sb, ysums, rows))
                return out

            # ---- attention pieces ----
            gstate = {}

            def emit_scores(ci, p):
                s = CST[ci]
                g = p // 2
                loc = p % 2
                s2 = psS.tile([CL, 1024], f32, tag="s2", name="s2")
                nc.tensor.matmul(
                    s2[:, 0:P],
                    lhsT=QT[p][0:64, s:s + CL],
                    rhs=KT[p][0:64, s:s + P],
                    start=True, stop=True,
                )
                nc.tensor.matmul(
                    s2[:, 512:512 + P],
                    lhsT=QT[p][64:128, s:s + CL],
                    rhs=KT[p][64:128, s:s + P],
                    start=True, stop=True,
                )
                if loc == 0:
                    pm = pmp.tile([CL, 512], f32, tag="pm", name="pm")
                    gstate[(ci, g)] = {"pm": pm}
                pm = gstate[(ci, g)]["pm"]
                nc.vector.scalar_tensor_tensor(
                    out=pm.rearrange(
                        "a (h w) -> a h w", h=4)[:, 2 * loc:2 * loc + 2, :],
                    in0=s2.rearrange("a (b w) -> a b w", b=2)[:, :, 0:P],
                    scalar=1.0 / TEMP,
                    in1=msk_sb.rearrange("a (h w) -> a h w", h=2),
                    op0=Alu.mult, op1=Alu.add,
                )

            def emit_softmax(ci, g):
                st = gstate[(ci, g)]
                pm = st["pm"]
                pe = pep.tile([CL, 512], bf16, tag="pe", name="pe")
                nc.scalar.activation(out=pe, in_=pm, func=Act.Exp)
                rs = small.tile([CL, 4], f32, tag="rs", name="rs")
                nc.vector.tensor_reduce(
                    out=rs, in_=pe.rearrange("a (h w) -> a h w", h=4),
                    axis=mybir.AxisListType.X, op=Alu.add,
                )
                rsr = small.tile([CL, 4], f32, tag="rsr", name="rsr")
                nc.vector.reciprocal(rsr, rs)
                pn = pnp.tile([CL, 512], bf16, tag="pn", name="pn")
                nc.gpsimd.tensor_tensor(
                    pn.rearrange("a (h w) -> a h w", h=4),
                    pe.rearrange("a (h w) -> a h w", h=4),
                    rsr[:, :, None].to_broadcast((CL, 4, P)),
                    Alu.mult,
                )
                st["pn"] = pn

            def emit_tpv(ci, g):
                st = gstate.pop((ci, g))
                pn4 = st["pn"].rearrange("a (h w) -> a h w", h=4)
                s = CST[ci]
                pt = psT.tile([P, 4 * CL], bf16, tag="pt", name="pt")
                for h in range(4):
                    nc.tensor.transpose(
                        pt[:, h * CL:(h + 1) * CL], pn4[:, h, :], idn_sb,
                    )
                ptsb = ptp.tile([P, 4 * CL], bf16, tag="ptsb", name="ptsb")
                nc.vector.tensor_copy(ptsb, pt)
                vt = V_tiles[ci]
                ot = psO.tile([P, 2 * CL], f32, tag="ot", name="ot")
                for j in (0, 1):
                    pair = 2 * g + j
                    js = slice(j * CL, (j + 1) * CL)
                    nc.tensor.matmul(
                        ot[0:64, js],
                        lhsT=vt[:, pair * P:pair * P + 64],
                        rhs=ptsb[:, (2 * j) * CL:(2 * j + 1) * CL],
                        start=True, stop=True,
                    )
                    nc.tensor.matmul(
                        ot[64:128, js],
                        lhsT=vt[:, pair * P + 64:(pair + 1) * P],
                        rhs=ptsb[:, (2 * j + 1) * CL:(2 * j + 2) * CL],
                        start=True, stop=True,
                    )
                oview = OTr[:, 2 * g:2 * g + 2, s:s + CL]
                nc.scalar.activation(
                    out=oview,
                    in_=ot.rearrange("a (j w) -> a j w", j=2),
                    func=Act.Copy)

            # ---- the pipeline ----
            from collections import deque

            vq = deque()
            fq = deque()
            gfifo = deque()
            slot = 0

            for f in v_thunks(0):      # prologue: project V(0) densely
                f()

            for ci in range(NCH):
                if ci < NCH - 1:
                    vq.extend(v_thunks(ci + 1))
                if ci in FCMAP:
                    for item in FCMAP[ci]:
                        if isinstance(item, tuple):
                            fq.extend(fc_thunks(-1, t0=item[0],
                                                rows=item[1]))
                        else:
                            fq.extend(fc_thunks(item))
                for p in range(8):
                    emit_scores(ci, p)
                    if p % 2 == 1:
                        emit_softmax(ci, p // 2)
                        gfifo.append((ci, p // 2, slot))
                    nv = -(-len(vq) // (8 - p))
                    for _ in range(min(nv, 4)):
                        if vq:
                            vq.popleft()()
                    off_now = OFF if ci < NCH - 1 else 2
                    if gfifo and slot - gfifo[0][2] >= off_now:
                        gci, gg, _ = gfifo.popleft()
                        emit_tpv(gci, gg)
                    if p >= 2:
                        nf = -(-len(fq) // (8 - p))
                        if ci == NCH - 1:
                            nf = max(nf, 6)
                        for _ in range(min(nf, 6)):
                            if fq:
                                fq.popleft()()
                    slot += 1

            # ---- epilogue: drain pending groups, then final FC chunk ----
            while gfifo:
                gci, gg, _ = gfifo.popleft()
                emit_tpv(gci, gg)
            while fq:
                fq.popleft()()
            for f in fc_thunks(-1, t0=960, rows=64):
                f()

    nc.compile()
    return nc


def _get_program(apply_affine: bool):
    key = ("prog", apply_affine)
    if key not in _CACHE:
        _CACHE[key] = _build_program(apply_affine)
    return _CACHE[key]


def _host_prep(inputs):
    x = np.asarray(inputs["x"], np.float32)
    xs = np.asarray(inputs["xs"], np.float32)
    w_qs = np.asarray(inputs["w_qs"], np.float32)
    b_qs = np.asarray(inputs["b_qs"], np.float32)
    w_ks = np.asarray(inputs["w_ks"], np.float32)
    w_vs = np.asarray(inputs["w_vs"], np.float32)
    b_vs = np.asarray(inputs["b_vs"], np.float32)
    w_fc = np.asarray(inputs["w_fc"], np.float32)
    b_fc = np.asarray(inputs["b_fc"], np.float32)
    ln_g = np.asarray(inputs["ln_g"], np.float32)
    ln_b = np.asarray(inputs["ln_b"], np.float32)

    apply_affine = not (np.all(ln_g == 1.0) and np.all(ln_b == 0.0))

    bprime = (b_vs @ w_fc + b_fc).astype(np.float32)

    mask = np.full((CL, P), NEG, np.float32)
    for t in range(CL):
        mask[t, t:t + 2 * NEI + 1] = 0.0
    mask2 = np.concatenate([mask, mask], axis=1)

    shared = {
        "wq": np.ascontiguousarray(w_qs.astype(BF16)),
        "wk": np.ascontiguousarray(w_ks.astype(BF16)),
        "wv": np.ascontiguousarray(w_vs.astype(BF16)),
        "wf": np.ascontiguousarray(w_fc.astype(BF16)),
        "bq": np.ascontiguousarray(b_qs.reshape(ND, P).T.astype(np.float32)),
        "msk": np.ascontiguousarray(mask2),
        "idn": np.eye(CL, dtype=BF16),
    }
    if apply_affine:
        shared["lng"] = np.ascontiguousarray(ln_g.reshape(1, D))
        shared["lnb"] = np.ascontiguousarray(ln_b.reshape(1, D))

    in_maps = []
    half_n = S // 2  # 1024
    for core in range(NCORES):
        b, half = core // 2, core % 2
        t0 = half * half_n
        xqc = x[b, t0:t0 + half_n] + bprime[None, :]
        halo = np.zeros((TH, D), np.float32)
        lo = max(0, t0 - NEI)
        hi = min(S, t0 + half_n + NEI)
        halo[lo - (t0 - NEI):hi - (t0 - NEI)] = xs[b, lo:hi]
        m = dict(shared)
        m["xq"] = np.ascontiguousarray(xqc.astype(BF16))
        m["xqT"] = np.ascontiguousarray(x[b, t0:t0 + half_n].T.astype(BF16))
        m["xsT"] = np.ascontiguousarray(halo.T.astype(BF16))
        in_maps.append(m)
    return in_maps, apply_affine


def _run(inputs, trace=False, trace_kwargs=None):
    from concourse.bass_utils import run_bass_kernel_spmd

    in_maps, apply_affine = _host_prep(inputs)
    nc = _get_program(apply_affine)
    res = run_bass_kernel_spmd(
        nc, in_maps, list(range(NCORES)),
        trace=trace, **(trace_kwargs or {})
    )
    y = np.empty((B, S, D), np.float32)
    half_n = S // 2
    for core in range(NCORES):
        b, half = core // 2, core % 2
        y[b, half * half_n:(half + 1) * half_n] = np.asarray(
            res.results[core]["yo"], dtype=np.float32)
    return y, res


def kernel(**inputs):
    y, _ = _run(inputs)
    return y


# revision 32
# speedup vs baseline: 1.0325x; 1.0325x over previous
"""LocalSelfAttention (window=7) Trainium2 Bass kernel — pipelined v2.

Full inputs in, full output out. Sharding: 8 cores = batch(4) x seq-half(2),
each core handles 1024 tokens with a 3-token zero-padded halo on xs.

Math notes (exact rewrites of the reference):
- reference projects zero-PADDED xs patches, so out-of-range taps have
  k = b_ks, v = b_vs. Softmax over taps is invariant to the per-(t,h)
  constant q . b_ks, so the K bias drops entirely; softmax weights sum to 1,
  so the V bias contributes exactly b_vs to o. Both b_vs @ w_fc and b_fc are
  folded into the residual on the host: xq = x + b_vs @ w_fc + b_fc.

Structure (per core):
- Q projection (dc-outer, 8 psum banks) -> QT feature-major bf16.
- K projection over the 1056-wide halo in 3 stripes -> KT feature-major.
- Software-pipelined chunk loop over 11 chunks of 96 tokens:
  PE issue order interleaves score matmuls (chunk ci), V projection
  (chunk ci+1), prob-transposes + PV (chunk ci, delayed 3 pair-slots),
  and FC matmuls (128-token chunks, dependency-mapped) so the tensor
  engine never sits behind the softmax chain.  Elementwise work is spread
  over DVE (mask+scale stt, reduces, recip, pt-copy, FC residual-add),
  ACT (exp, V/OT evictions, LN squares + final scale), and Pool/GpSimd
  (prob normalization, LN scalar chain) - Pool cannot touch PSUM.
"""

import sys

for _p in ("/opt/trn_rl_repo",):
    if _p not in sys.path:
        sys.path.insert(0, _p)

import numpy as np
import ml_dtypes

BF16 = ml_dtypes.bfloat16

H, DK, DV, D = 16, 64, 64, 1024
NEI = 3
TEMP = 8.0
EPS = 1e-5
B, S = 4, 2048
NCORES = 8
T = (B * S) // NCORES          # 1024 tokens per core
TH = T + 2 * NEI               # 1030 halo tokens
P = 128
NT = T // P                    # 8 fc-phase token chunks (128 tokens)
ND = D // P                    # 8 feature chunks
CL = 96                        # attention chunk length
NCH = 11
CST = [96 * i for i in range(10)] + [928]          # attn chunk starts
TH2 = 1056                     # padded halo width
WB = 104                       # live score width (CL + 6 window, padded)
NEG = -30000.0
RSQRT_MAGIC = 0x5F3759DF       # Quake rsqrt seed; 2 Newton steps follow
KSTRIPES = [(0, 384), (384, 384), (768, 288)]      # K projection stripes
# FC chunk c (tokens 128c..128c+128) is emitted during attn chunk FCMAP^-1:
# dep(c) = first attn chunk index ci with 96ci+96 >= 128c+128; emit at dep+1.
FCMAP = {2: 0, 3: 1, 4: 2, 6: 3, 7: 4, 8: 5, 10: 6}   # ci -> fc chunk
OFF = 4                        # pair-slots between scores and transpose/PV

_CACHE = {}


def _build_program(apply_affine: bool):
    import concourse.bacc as bacc
    import concourse.tile as tile
    from concourse import mybir
    from contextlib import ExitStack

    f32 = mybir.dt.float32
    bf16 = mybir.dt.bfloat16
    Alu = mybir.AluOpType
    Act = mybir.ActivationFunctionType

    nc = bacc.Bacc(
        "TRN2", target_bir_lowering=False, debug=False, enable_asserts=False
    )

    def din(name, shape, dt_):
        return nc.dram_tensor(name, shape, dt_, kind="ExternalInput").ap()

    xq = din("xq", (T, D), bf16)         # residual + folded fc/v bias
    xqT = din("xqT", (D, T), bf16)       # x^T (host-transposed)
    xsT = din("xsT", (D, TH), bf16)      # xs^T with halo (host-transposed)
    wq = din("wq", (D, D), bf16)
    wk = din("wk", (D, D), bf16)
    wv = din("wv", (D, D), bf16)
    wf = din("wf", (D, D), bf16)
    bq = din("bq", (P, ND), f32)         # b_qs laid out [p, ec]
    msk = din("msk", (CL, 2 * P), f32)   # band mask 0 / NEG, two head slots
    idn = din("idn", (CL, CL), bf16)     # identity for PE transpose
    if apply_affine:
        lng = din("lng", (1, D), f32)
        lnb = din("lnb", (1, D), f32)
    yo = nc.dram_tensor("yo", (T, D), f32, kind="ExternalOutput").ap()

    with tile.TileContext(nc) as tc, ExitStack() as ctx:
        consts = ctx.enter_context(tc.tile_pool(name="consts", bufs=1))
        big = ctx.enter_context(tc.tile_pool(name="big", bufs=1))
        vpool = ctx.enter_context(tc.tile_pool(name="vpool", bufs=3))
        pmp = ctx.enter_context(tc.tile_pool(name="pmp", bufs=2))
        pep = ctx.enter_context(tc.tile_pool(name="pep", bufs=3))
        pnp = ctx.enter_context(tc.tile_pool(name="pnp", bufs=3))
        ptp = ctx.enter_context(tc.tile_pool(name="ptp", bufs=2))
        small = ctx.enter_context(tc.tile_pool(name="small", bufs=4))
        lnpool = ctx.enter_context(tc.tile_pool(name="lnpool", bufs=2))

        # ---- constants ----
        msk_sb = consts.tile([CL, 2 * P], f32, tag="msk")
        nc.sync.dma_start(out=msk_sb, in_=msk)
        idn_sb = consts.tile([CL, CL], bf16, tag="idn")
        nc.sync.dma_start(out=idn_sb, in_=idn)
        bq_sb = consts.tile([P, ND], f32, tag="bq")
        nc.sync.dma_start(out=bq_sb, in_=bq)
        magic_sb = consts.tile([P, 1], mybir.dt.uint32, tag="magic")
        nc.vector.memset(magic_sb, RSQRT_MAGIC)
        if apply_affine:
            import concourse.bass as bass

            g_bc = consts.tile([P, D], f32, tag="g_bc")
            b_bc = consts.tile([P, D], f32, tag="b_bc")
            nc.sync.dma_start(
                out=g_bc,
                in_=bass.AP(tensor=lng.tensor, offset=lng.offset,
                            ap=[[0, P]] + list(lng.ap[1:])),
            )
            nc.sync.dma_start(
                out=b_bc,
                in_=bass.AP(tensor=lnb.tensor, offset=lnb.offset,
                            ap=[[0, P]] + list(lnb.ap[1:])),
            )

        # ---- input loads: (wq,xT) pairs first so Q proj can start early ----
        wq_t, xT_t = [], []
        for dc in range(ND):
            w1 = big.tile([P, D], bf16, tag=f"wq{dc}", name=f"wq{dc}")
            nc.sync.dma_start(out=w1, in_=wq[dc * P:(dc + 1) * P, :])
            wq_t.append(w1)
            t1 = big.tile([P, T], bf16, tag=f"xT{dc}", name=f"xT{dc}")
            nc.sync.dma_start(out=t1, in_=xqT[dc * P:(dc + 1) * P, :])
            xT_t.append(t1)
        wk_t, xsT_t = [], []
        for dc in range(ND):
            w2 = big.tile([P, D], bf16, tag=f"wk{dc}", name=f"wk{dc}")
            nc.sync.dma_start(out=w2, in_=wk[dc * P:(dc + 1) * P, :])
            wk_t.append(w2)
            t2 = big.tile([P, TH2], bf16, tag=f"xsT{dc}", name=f"xsT{dc}")
            nc.sync.dma_start(out=t2[:, 0:TH], in_=xsT[dc * P:(dc + 1) * P, :])
            nc.vector.memset(t2[:, TH:TH2], 0.0)
            xsT_t.append(t2)
        wv_t, wf_t = [], []
        for dc in range(ND):
            w3 = big.tile([P, D], bf16, tag=f"wv{dc}", name=f"wv{dc}")
            nc.sync.dma_start(out=w3, in_=wv[dc * P:(dc + 1) * P, :])
            wv_t.append(w3)
        for dc in range(ND):
            w4 = big.tile([P, D], bf16, tag=f"wf{dc}", name=f"wf{dc}")
            nc.sync.dma_start(out=w4, in_=wf[dc * P:(dc + 1) * P, :])
            wf_t.append(w4)
        xr_t = []
        for c in range(NT):
            xr = big.tile([P, D], bf16, tag=f"xr{c}", name=f"xr{c}")
            nc.sync.dma_start(out=xr, in_=xq[c * P:(c + 1) * P, :])
            xr_t.append(xr)

        QT = [big.tile([P, T], bf16, tag=f"QT{ec}", name=f"QT{ec}")
              for ec in range(ND)]
        KT = [big.tile([P, TH2], bf16, tag=f"KT{ec}", name=f"KT{ec}")
              for ec in range(ND)]
        OT_all = big.tile([P, ND * T], bf16, tag="OT_all", name="OT_all")
        OTr = OT_all.rearrange("a (p w) -> a p w", p=ND)

        # ---- Q projection: dc-outer, 8 psum banks, ACT evict w/ bias ----
        with tc.tile_pool(name="psP", bufs=1, space="PSUM") as psP:
            for half in (0, 1):
                hs = slice(half * 512, (half + 1) * 512)
                ps = [psP.tile([P, 512], f32, tag=f"pp{ec}", name=f"pp{ec}")
                      for ec in range(ND)]
                for dc in range(ND):
                    for ec in range(ND):
                        nc.tensor.matmul(
                            ps[ec],
                            lhsT=wq_t[dc][:, ec * P:(ec + 1) * P],
                            rhs=xT_t[dc][:, hs],
                            start=(dc == 0), stop=(dc == ND - 1),
                        )
                for ec in range(ND):
                    nc.scalar.activation(out=QT[ec][:, hs], in_=ps[ec],
                                         func=Act.Identity,
                                         bias=bq_sb[:, ec:ec + 1], scale=1.0)
            # ---- K projection: 3 stripes over the 1056 halo, DVE evict ----
            for s0, w in KSTRIPES:
                ps = [psP.tile([P, 384], f32, tag=f"pp{ec}", name=f"pk{ec}")
                      for ec in range(ND)]
                for dc in range(ND):
                    for ec in range(ND):
                        nc.tensor.matmul(
                            ps[ec][:, 0:w],
                            lhsT=wk_t[dc][:, ec * P:(ec + 1) * P],
                            rhs=xsT_t[dc][:, s0:s0 + w],
                            start=(dc == 0), stop=(dc == ND - 1),
                        )
                for ec in range(ND):
                    nc.vector.tensor_copy(KT[ec][:, s0:s0 + w], ps[ec][:, 0:w])

        # ---- pipelined chunk loop ----
        with tc.tile_pool(name="psS", bufs=2, space="PSUM") as psS, \
             tc.tile_pool(name="psV", bufs=1, space="PSUM") as psV, \
             tc.tile_pool(name="psT", bufs=1, space="PSUM") as psT, \
             tc.tile_pool(name="psO", bufs=1, space="PSUM") as psO, \
             tc.tile_pool(name="psF", bufs=1, space="PSUM") as psF:

            V_tiles = {}

            def v_thunks(ci):
                """16 matmuls + 2 evictions projecting V for chunk ci."""
                s = CST[ci]
                vt = vpool.tile([P, D], bf16, tag="vt", name=f"V{ci}")
                V_tiles[ci] = vt
                out = []
                for half in (0, 1):
                    hs = slice(half * 512, (half + 1) * 512)
                    pv = [None]

                    def mk(dc, half=half, hs=hs, pv=pv, s=s, vt=vt):
                        def f():
                            if dc == 0:
                                pv[0] = psV.tile([P, 512], f32, tag="pv",
                                                 name="ps_v")
                            nc.tensor.matmul(
                                pv[0],
                                lhsT=xsT_t[dc][:, s:s + P],
                                rhs=wv_t[dc][:, hs],
                                start=(dc == 0), stop=(dc == ND - 1),
                            )
                            if dc == ND - 1:
                                nc.scalar.activation(out=vt[:, hs], in_=pv[0],
                                                     func=Act.Copy)
                        return f
                    out.extend(mk(dc) for dc in range(ND))
                return out

            def ln_block(c, y_sb, ysums):
                """LayerNorm for FC chunk c from y_sb; emits + output DMA.

                rstd = exp(-0.5*ln(var+eps)) keeps the ACT engine inside one
                activation table (natural_log_exp_and_others: copy/identity/
                exp/ln/square) - a Sqrt would force a 1.3us table reload.
                """
                cs = slice(c * P, (c + 1) * P)
                ysum = small.tile([P, 1], f32, tag="ysum", name="ysum")
                nc.gpsimd.tensor_add(ysum, ysums[0], ysums[1])
                sq0 = small.tile([P, 1], f32, tag="sq0", name="sq0")
                sq1 = small.tile([P, 1], f32, tag="sq1", name="sq1")
                ysq = lnpool.tile([P, 512], f32, tag="ysq", name="ysq")
                nc.scalar.activation(out=ysq, in_=y_sb[:, 0:512],
                                     func=Act.Square, accum_out=sq0)
                nc.scalar.activation(out=ysq, in_=y_sb[:, 512:1024],
                                     func=Act.Square, accum_out=sq1)
                ssum = small.tile([P, 1], f32, tag="ssum", name="ssum")
                nc.gpsimd.tensor_add(ssum, sq0, sq1)
                mean = small.tile([P, 1], f32, tag="mean", name="mean")
                nc.gpsimd.tensor_scalar_mul(mean, ysum, 1.0 / D)
                msq = small.tile([P, 1], f32, tag="msq", name="msq")
                nc.gpsimd.tensor_mul(msq, mean, mean)
                # veps = ssum/D - msq + EPS, then rstd = veps^-1/2 via the
                # bit-trick seed + 2 Newton steps, all on GpSimd (no cross-
                # engine ping-pong, and no Sqrt/Ln on ACT - those live in a
                # different activation table and force 1.3us table reloads).
                msqme = small.tile([P, 1], f32, tag="msqme", name="msqme")
                nc.gpsimd.tensor_scalar_sub(msqme, msq, EPS)
                sdiv = small.tile([P, 1], f32, tag="sdiv", name="sdiv")
                nc.gpsimd.tensor_scalar_mul(sdiv, ssum, 1.0 / D)
                veps = small.tile([P, 1], f32, tag="veps", name="veps")
                nc.gpsimd.tensor_sub(veps, sdiv, msqme)
                u32 = mybir.dt.uint32
                sh_u = small.tile([P, 1], u32, tag="sh_u", name="sh_u")
                nc.vector.tensor_scalar(
                    out=sh_u, in0=veps.bitcast(u32), scalar1=1, scalar2=None,
                    op0=Alu.logical_shift_right,
                )
                sb_u = small.tile([P, 1], u32, tag="sb_u", name="sb_u")
                nc.vector.tensor_tensor(sb_u, magic_sb, sh_u, Alu.subtract)
                r = sb_u.bitcast(f32)
                for it in range(2):
                    t1 = small.tile([P, 1], f32, tag=f"nr{it}a", name="nr_a")
                    nc.vector.tensor_mul(t1, r, r)
                    nc.vector.tensor_mul(t1, veps, t1)
                    nc.vector.tensor_scalar(
                        out=t1, in0=t1, scalar1=-0.5, scalar2=1.5,
                        op0=Alu.mult, op1=Alu.add,
                    )
                    rn = small.tile([P, 1], f32, tag=f"nr{it}b", name="nr_b")
                    nc.vector.tensor_mul(rn, r, t1)
                    r = rn
                rstd = r
                bact = small.tile([P, 1], f32, tag="bact", name="bact")
                nc.vector.tensor_mul(bact, mean, rstd)
                nc.vector.tensor_scalar_mul(bact, bact, -1.0)
                out_sb = lnpool.tile([P, D], f32, tag="osb", name="out_sb")
                nc.scalar.activation(out=out_sb, in_=y_sb, func=Act.Identity,
                                     bias=bact, scale=rstd)
                if apply_affine:
                    nc.vector.tensor_mul(out_sb, out_sb, g_bc)
                    nc.vector.tensor_add(out_sb, out_sb, b_bc)
                nc.sync.dma_start(out=yo[cs, :], in_=out_sb)

            def fc_thunks(c):
                """16 matmuls + 2 residual-stts + LN for 128-token chunk c."""
                cs = slice(c * P, (c + 1) * P)
                y_sb = lnpool.tile([P, D], f32, tag="ysb", name="y_sb")
                ysums = [None, None]
                out = []
                for half in (0, 1):
                    hs = slice(half * 512, (half + 1) * 512)
                    pf = [None]

                    def mk(ec, half=half, hs=hs, pf=pf, cs=cs, y_sb=y_sb,
                           c=c):
                        def f():
                            if ec == 0:
                                pf[0] = psF.tile([P, 512], f32, tag="pf",
                                                 name="ps_f")
                            nc.tensor.matmul(
                                pf[0],
                                lhsT=OTr[:, ec, cs],
                                rhs=wf_t[ec][:, hs],
                                start=(ec == 0), stop=(ec == ND - 1),
                            )
                            if ec == ND - 1:
                                ysums[half] = small.tile(
                                    [P, 1], f32, tag=f"ysm{half}",
                                    name="ysm")
                                nc.vector.scalar_tensor_tensor(
                                    out=y_sb[:, hs], in0=pf[0], scalar=1.0,
                                    in1=xr_t[c][:, hs],
                                    op0=Alu.mult, op1=Alu.add,
                                    accum_out=ysums[half],
                                )
                        return f
                    out.extend(mk(ec) for ec in range(ND))
                out.append(lambda: ln_block(c, y_sb, ysums))
                return out

            # ---- attention pieces ----
            gstate = {}

            def emit_scores(ci, p):
                s = CST[ci]
                g = p // 2
                loc = p % 2
                s2 = psS.tile([CL, 1024], f32, tag="s2", name="s2")
                nc.tensor.matmul(
                    s2[:, 0:P],
                    lhsT=QT[p][0:64, s:s + CL],
                    rhs=KT[p][0:64, s:s + P],
                    start=True, stop=True,
                )
                nc.tensor.matmul(
                    s2[:, 512:512 + P],
                    lhsT=QT[p][64:128, s:s + CL],
                    rhs=KT[p][64:128, s:s + P],
                    start=True, stop=True,
                )
                if loc == 0:
                    pm = pmp.tile([CL, 512], f32, tag="pm", name="pm")
                    gstate[(ci, g)] = {"pm": pm}
                pm = gstate[(ci, g)]["pm"]
                nc.vector.scalar_tensor_tensor(
                    out=pm.rearrange(
                        "a (h w) -> a h w", h=4)[:, 2 * loc:2 * loc + 2, :],
                    in0=s2.rearrange("a (b w) -> a b w", b=2)[:, :, 0:P],
                    scalar=1.0 / TEMP,
                    in1=msk_sb.rearrange("a (h w) -> a h w", h=2),
                    op0=Alu.mult, op1=Alu.add,
                )

            def emit_softmax(ci, g):
                st = gstate[(ci, g)]
                pm = st["pm"]
                pe = pep.tile([CL, 512], bf16, tag="pe", name="pe")
                nc.scalar.activation(out=pe, in_=pm, func=Act.Exp)
                rs = small.tile([CL, 4], f32, tag="rs", name="rs")
                nc.vector.tensor_reduce(
                    out=rs, in_=pe.rearrange("a (h w) -> a h w", h=4),
                    axis=mybir.AxisListType.X, op=Alu.add,
                )
                rsr = small.tile([CL, 4], f32, tag="rsr", name="rsr")
                nc.vector.reciprocal(rsr, rs)
                pn = pnp.tile([CL, 512], bf16, tag="pn", name="pn")
                nc.gpsimd.tensor_tensor(
                    pn.rearrange("a (h w) -> a h w", h=4),
                    pe.rearrange("a (h w) -> a h w", h=4),
                    rsr[:, :, None].to_broadcast((CL, 4, P)),
                    Alu.mult,
                )
                st["pn"] = pn

            def emit_tpv(ci, g):
                st = gstate.pop((ci, g))
                pn4 = st["pn"].rearrange("a (h w) -> a h w", h=4)
                s = CST[ci]
                pt = psT.tile([P, 4 * CL], bf16, tag="pt", name="pt")
                for h in range(4):
                    nc.tensor.transpose(
                        pt[:, h * CL:(h + 1) * CL], pn4[:, h, :], idn_sb,
                    )
                ptsb = ptp.tile([P, 4 * CL], bf16, tag="ptsb", name="ptsb")
                nc.vector.tensor_copy(ptsb, pt)
                vt = V_tiles[ci]
                ot = psO.tile([P, 2 * CL], f32, tag="ot", name="ot")
                for j in (0, 1):
                    pair = 2 * g + j
                    js = slice(j * CL, (j + 1) * CL)
                    nc.tensor.matmul(
                        ot[0:64, js],
                        lhsT=vt[:, pair * P:pair * P + 64],
                        rhs=ptsb[:, (2 * j) * CL:(2 * j + 1) * CL],
                        start=True, stop=True,
                    )
                    nc.tensor.matmul(
                        ot[64:128, js],
                        lhsT=vt[:, pair * P + 64:(pair + 1) * P],
                        rhs=ptsb[:, (2 * j + 1) * CL:(2 * j + 2) * CL],
                        start=True, stop=True,
                    )
                oview = OTr[:, 2 * g:2 * g + 2, s:s + CL]
                nc.scalar.activation(
                    out=oview,
                    in_=ot.rearrange("a (j w) -> a j w", j=2),
                    func=Act.Copy)

            # ---- the pipeline ----
            from collections import deque

            vq = deque()
            fq = deque()
            gfifo = deque()
            slot = 0

            for f in v_thunks(0):      # prologue: project V(0) densely
                f()

            for ci in range(NCH):
                if ci < NCH - 1:
                    vq.extend(v_thunks(ci + 1))
                if ci in FCMAP:
                    fq.extend(fc_thunks(FCMAP[ci]))
                for p in range(8):
                    emit_scores(ci, p)
                    if p % 2 == 1:
                        emit_softmax(ci, p // 2)
                        gfifo.append((ci, p // 2, slot))
                    for _ in range(3):
                        if vq:
                            vq.popleft()()
                    if gfifo and slot - gfifo[0][2] >= OFF:
                        gci, gg, _ = gfifo.popleft()
                        emit_tpv(gci, gg)
                    if p >= 2:
                        for _ in range(3):
                            if fq:
                                fq.popleft()()
                    slot += 1

            # ---- epilogue: drain pending groups, then final FC chunk ----
            while gfifo:
                gci, gg, _ = gfifo.popleft()
                emit_tpv(gci, gg)
            while fq:
                fq.popleft()()
            for f in fc_thunks(7):
                f()

    nc.compile()
    return nc


def _get_program(apply_affine: bool):
    key = ("prog", apply_affine)
    if key not in _CACHE:
        _CACHE[key] = _build_program(apply_affine)
    return _CACHE[key]


def _host_prep(inputs):
    x = np.asarray(inputs["x"], np.float32)
    xs = np.asarray(inputs["xs"], np.float32)
    w_qs = np.asarray(inputs["w_qs"], np.float32)
    b_qs = np.asarray(inputs["b_qs"], np.float32)
    w_ks = np.asarray(inputs["w_ks"], np.float32)
    w_vs = np.asarray(inputs["w_vs"], np.float32)
    b_vs = np.asarray(inputs["b_vs"], np.float32)
    w_fc = np.asarray(inputs["w_fc"], np.float32)
    b_fc = np.asarray(inputs["b_fc"], np.float32)
    ln_g = np.asarray(inputs["ln_g"], np.float32)
    ln_b = np.asarray(inputs["ln_b"], np.float32)

    apply_affine = not (np.all(ln_g == 1.0) and np.all(ln_b == 0.0))

    bprime = (b_vs @ w_fc + b_fc).astype(np.float32)

    mask = np.full((CL, P), NEG, np.float32)
    for t in range(CL):
        mask[t, t:t + 2 * NEI + 1] = 0.0
    mask2 = np.concatenate([mask, mask], axis=1)

    shared = {
        "wq": np.ascontiguousarray(w_qs.astype(BF16)),
        "wk": np.ascontiguousarray(w_ks.astype(BF16)),
        "wv": np.ascontiguousarray(w_vs.astype(BF16)),
        "wf": np.ascontiguousarray(w_fc.astype(BF16)),
        "bq": np.ascontiguousarray(b_qs.reshape(ND, P).T.astype(np.float32)),
        "msk": np.ascontiguousarray(mask2),
        "idn": np.eye(CL, dtype=BF16),
    }
    if apply_affine:
        shared["lng"] = np.ascontiguousarray(ln_g.reshape(1, D))
        shared["lnb"] = np.ascontiguousarray(ln_b.reshape(1, D))

    in_maps = []
    half_n = S // 2  # 1024
    for core in range(NCORES):
        b, half = core // 2, core % 2
        t0 = half * half_n
        xqc = x[b, t0:t0 + half_n] + bprime[None, :]
        halo = np.zeros((TH, D), np.float32)
        lo = max(0, t0 - NEI)
        hi = min(S, t0 + half_n + NEI)
        halo[lo - (t0 - NEI):hi - (t0 - NEI)] = xs[b, lo:hi]
        m = dict(shared)
        m["xq"] = np.ascontiguousarray(xqc.astype(BF16))
        m["xqT"] = np.ascontiguousarray(x[b, t0:t0 + half_n].T.astype(BF16))
        m["xsT"] = np.ascontiguousarray(halo.T.astype(BF16))
        in_maps.append(m)
    return in_maps, apply_affine


def _run(inputs, trace=False, trace_kwargs=None):
    from concourse.bass_utils import run_bass_kernel_spmd

    in_maps, apply_affine = _host_prep(inputs)
    nc = _get_program(apply_affine)
    res = run_bass_kernel_spmd(
        nc, in_maps, list(range(NCORES)),
        trace=trace, **(trace_kwargs or {})
    )
    y = np.empty((B, S, D), np.float32)
    half_n = S // 2
    for core in range(NCORES):
        b, half = core // 2, core % 2
        y[b, half * half_n:(half + 1) * half_n] = res.results[core]["yo"]
    return y, res


def kernel(**inputs):
    y, _ = _run(inputs)
    return y
